# revision 5
# baseline (speedup 1.0000x reference)
"""Trainium2 Bass kernel for nn_Cov_EBFLayer (v3: host-prepped pair coeffs).

Math: out[b,o] = exp(-quad[o,b]),
  quad[o,b] = diff^T P_o diff,  diff = c_o - x_b,  P_o = B_o B_o^T
            = x^T P x - 2 v_o^T x + q3_o,   v = P c,  q3 = c^T P c
Square trick + rotation packing:
  x^T P x = sum_{d, g=1..32} s2_g * P[d, (d+g)%64] * (x_d + x_{(d+g)%64})^2
            - sum_d (r_d - 2 P_dd) x_d^2
  with s2_g = 1 for g<32, 1/2 for g=32; the 2080 unique pair features pack
  into 16 chunks of 128 rows (gj,d), g = 2c+1+gj.

Host prep gathers the rotated pair-coefficient matrix W[h][(gj,d),(c,o)] =
P_o[d,(d+g)%64] directly (same byte volume as the raw betas input), so the
device runs only: build features (indicator matmuls + ACT squares) and the
accumulating mains, per-chunk interleaved across both o-halves with all
four PSUM accumulators live; exp + output DMA staggered per (half, bt).
"""

import sys
from contextlib import ExitStack

import numpy as np

sys.path.insert(0, "/opt/trn_rl_repo")

import concourse.bass as bass  # noqa: E402
import concourse.tile as tile  # noqa: E402
from concourse import bacc, mybir  # noqa: E402
from concourse import bass_utils  # noqa: E402
from concourse._compat import with_exitstack  # noqa: E402

B, D, O, NCORES = 8192, 64, 256, 8
BSH = B // NCORES  # 1024 per-core batch shard
NCH = 16  # rotation-packed chunks: g = 2c+1+gj, rows (gj, d)
BT = 512  # b-tile (one PSUM bank of fp32)
NBT = BSH // BT  # 2
F32 = mybir.dt.float32
F16 = mybir.dt.float16
SQUARE = mybir.ActivationFunctionType.Square
EXP = mybir.ActivationFunctionType.Exp


@with_exitstack
def _kernel(ctx: ExitStack, tc, outT, xind, w0in, w1in, waug1, q3b):
    nc = tc.nc

    cpool = ctx.enter_context(tc.tile_pool(name="const", bufs=1))
    ppool = ctx.enter_context(tc.tile_pool(name="psum_p", bufs=4, space="PSUM"))
    qpool = ctx.enter_context(tc.tile_pool(name="psum_q", bufs=4, space="PSUM"))

    # ---- SBUF residents ----
    warm = cpool.tile([128, BT], F16)  # warmup scratch (memset)
    sb_xind = cpool.tile([D, BSH + NCH * 128], F16)  # [x | indicator]
    aug1 = cpool.tile([128, BSH], F16)  # [xT; x^2]
    sb_waug1 = cpool.tile([128, O], F16)  # [-2 v^T; (-r+2Pdd)^T]
    sb_q3b = cpool.tile([128, 2], F32)  # -q3 per (o-half) column
    w = [cpool.tile([128, NCH * 128], F16, name=f"w{h}") for h in range(2)]
    gstore = cpool.tile([128, NCH * NBT * BT], F16)  # squared features
    osb = [cpool.tile([128, BSH], F16, name=f"osb{h}") for h in range(2)]

    # ---- input DMA on the two HWDGE queues; x + indc2 gate builds so
    # they go first; W pieces in consumption order (h0 then h1) ----
    # x and the indicator ride ONE DMA (one completion semaphore, one
    # ring slot): each extra DMA on the gating chain costs a ~0.9us ring
    # hole plus a ~1.4us completion-semaphore lag before the PE sees it
    nc.sync.dma_start(sb_xind[:], xind[:])
    nc.sync.dma_start(sb_waug1[:], waug1[:])
    for h, win in ((0, w0in), (1, w1in)):
        nc.sync.dma_start(w[h][:, 0:1024], win[:, 0:1024])
        nc.scalar.dma_start(w[h][:, 1024:2048], win[:, 1024:2048])
    # warm memset on gpsimd (its queue opens earliest; vector would add
    # ~0.5us before the first warm matmul)
    nc.gpsimd.memset(warm[:], 0.125)
    # gpsimd (SWDGE) queue: small inputs needed only late (mains/exp)
    nc.gpsimd.dma_start(sb_q3b[:], q3b[:])

    # mains accumulators: all four (h, bt) live at once (4 PSUM banks)
    pq = {}
    for h in range(2):
        for bt in range(NBT):
            pq[(h, bt)] = qpool.tile([128, BT], F32, name=f"pq_{h}_{bt}", tag="pq")

    # ---- PE warmup: bridge GAPLESSLY until indc2/x land.  The HAM clock
    # gate promotes 1.2->2.4GHz only after a fully-busy free-running
    # 3.4us window; any idle gap here pushes full clock out by several
    # microseconds.  Coarse 512-col matmuls cover the bulk, short
    # 128-col ones give a fine-grained hand-off to the first build ----
    # 16 back-to-back FULL-ARRAY (K=128) 512-col warmups, ~6.8us gapless
    # at 1.2GHz: the HAM clock gate only credits full-row matmul activity
    # (K=64 work never promotes it), so a guaranteed-busy window here
    # locks in 2.4GHz before any real work issues; once promoted, the
    # remaining warmups run 2x faster, self-compensating
    wps = ppool.tile([128, BT], F32, name="wps", tag="u")
    for i in range(12):
        nc.tensor.matmul(
            wps[:], warm[0:128, 0:128], warm[0:128, 0:BT], start=True, stop=True
        )

    # ---- aug chunk rows: x copied + x^2 (DVE; off the critical path,
    # only the late aug matmuls read aug1) ----
    nc.vector.tensor_copy(aug1[0:64, :], sb_xind[:, 0:BSH])
    for bt in range(NBT):
        nc.vector.tensor_mul(
            aug1[64:128, bt * BT : (bt + 1) * BT],
            sb_xind[0:64, bt * BT : (bt + 1) * BT],
            sb_xind[0:64, bt * BT : (bt + 1) * BT],
        )

    # aug matmuls open each accumulation group (start=True): they are
    # K=128 real work, so they extend the promotion stream in place of
    # more junk warmups AND drop out of the kernel tail entirely
    for h in range(2):
        for bt in range(NBT):
            nc.tensor.matmul(
                pq[(h, bt)],
                sb_waug1[:, h * 128 : (h + 1) * 128],
                aug1[:, bt * BT : (bt + 1) * BT],
                start=True,
                stop=False,
            )

    # ---- builds + mains, per-chunk interleaved (mains trail by TRAIL so
    # the PE queue never head-blocks on the W input stream) ----
    # squares split per b-tile across ACT (bt0) and a DVE copy+multiply
    # pair (bt1): one engine cannot keep up with a full-clock PE, which
    # would make the whole mid-kernel PSUM-evacuation-bound.  1-bank bd
    # tiles (4-deep rotation) give the recycle slack.  (tensor_tensor
    # cannot read PSUM for both operands, hence the DVE copy.)
    stg = [cpool.tile([128, BT], F16, name=f"stg{i}") for i in range(2)]

    def build_chunk(c):
        for bt in range(NBT):
            bd = ppool.tile([128, BT], F32, name=f"bd_{c}_{bt}", tag="u")
            nc.tensor.matmul(
                bd[:],
                sb_xind[:, BSH + c * 128 : BSH + (c + 1) * 128],
                sb_xind[0:D, bt * BT : (bt + 1) * BT],
                start=True,
                stop=True,
            )
            gsl = gstore[:, (c * NBT + bt) * BT : (c * NBT + bt + 1) * BT]
            if bt == 0:
                nc.scalar.activation(gsl, bd[:], SQUARE)
            else:
                st = stg[c % 2]
                nc.vector.tensor_copy(st[:], bd[:])
                nc.vector.tensor_mul(gsl, st[:], st[:])

    def mains_pair(h, c0):
        # per accumulator, two consecutive chunks back-to-back: repeated
        # matmuls into the SAME PSUM bank with changing stationaries is
        # the pattern that streams at the 216ns/512-col roofline (the
        # completion wave proves it); alternating banks with a held
        # stationary exposes ~95ns per matmul instead
        for bt in range(NBT):
            for c in (c0, c0 + 1):
                nc.tensor.matmul(
                    pq[(h, bt)],
                    w[h][:, c * 128 : (c + 1) * 128],
                    gstore[:, (c * NBT + bt) * BT : (c * NBT + bt + 1) * BT],
                    start=False,
                    stop=False,
                )

    TRAIL = 3  # in chunk pairs
    CWAVE = 2  # final chunks handled per-accumulator in the completion wave
    NP = NCH // 2
    for cc in range(NP + TRAIL):
        if cc < NP:
            build_chunk(2 * cc)
            build_chunk(2 * cc + 1)
        if cc >= TRAIL:
            c0 = 2 * (cc - TRAIL)
            if c0 >= NCH - CWAVE:
                break
            mains_pair(0, c0)
            mains_pair(1, c0)

    # completion wave: per accumulator, its last CWAVE chunk mains + aug,
    # then exp + output DMA — each epilogue overlaps the next
    # accumulator's matmuls instead of serializing at the very end
    for h, bt in ((0, 0), (0, 1), (1, 0), (1, 1)):
        for c in range(NCH - CWAVE, NCH):
            nc.tensor.matmul(
                pq[(h, bt)],
                w[h][:, c * 128 : (c + 1) * 128],
                gstore[:, (c * NBT + bt) * BT : (c * NBT + bt + 1) * BT],
                start=False,
                stop=(c == NCH - 1),
            )
        nc.scalar.activation(
            osb[h][:, bt * BT : (bt + 1) * BT],
            pq[(h, bt)],
            EXP,
            bias=sb_q3b[:, h : h + 1],
            scale=-1.0,
        )
        # first three outs issue from sync (a scalar-issued DMA would
        # queue in front of the next exp on ACT); the last issues from
        # scalar — no exp follows it, and sync's queue is still busy
        # with the third issue at that point
        eng = nc.scalar if (h, bt) == (1, 1) else nc.sync
        eng.dma_start(
            outT[h * 128 : (h + 1) * 128, bt * BT : (bt + 1) * BT],
            osb[h][:, bt * BT : (bt + 1) * BT],
        )


_CACHE = {}


def _build():
    if "nc" in _CACHE:
        return _CACHE["nc"], _CACHE["aps"]
    nc = bacc.Bacc(
        "TRN2", target_bir_lowering=False, debug=False, num_devices=NCORES
    )
    xind = nc.dram_tensor(
        "xind", [D, BSH + NCH * 128], F16, kind="ExternalInput"
    ).ap()
    w0in = nc.dram_tensor("w0in", [128, 2048], F16, kind="ExternalInput").ap()
    w1in = nc.dram_tensor("w1in", [128, 2048], F16, kind="ExternalInput").ap()
    waug1 = nc.dram_tensor("waug1", [128, O], F16, kind="ExternalInput").ap()
    q3b = nc.dram_tensor("q3b", [128, 2], F32, kind="ExternalInput").ap()
    outT = nc.dram_tensor("outT", [O, BSH], F16, kind="ExternalOutput").ap()
    with tile.TileContext(nc) as tc:
        _kernel(tc, outT, xind, w0in, w1in, waug1, q3b)
    nc.compile()
    _CACHE["nc"] = nc
    _CACHE["aps"] = (xind, w0in, w1in, waug1, q3b, outT)
    return nc, _CACHE["aps"]


def _host_prep(x, centers, betas):
    x = np.asarray(x, np.float32)
    betas = np.asarray(betas, np.float32)
    c = np.asarray(centers, np.float32).reshape(O, D)
    # P_o = B_o B_o^T, then gather the rotated pair-coefficient layout
    # W[h][(gj,d), (c, o128)] = P[h*128+o128][d, (d+2c+1+gj) % 64]
    P = np.einsum("ode,ofe->odf", betas, betas)  # [O, D, D]
    dd = np.arange(D)[:, None]  # d
    ccg = np.arange(NCH)[None, :]  # c
    wh = np.empty((2, 2, D, NCH, 128), np.float32)  # [h, gj, d, c, o128]
    for h in range(2):
        for gj in range(2):
            f = (dd + 2 * ccg + 1 + gj) % D  # [d, c]
            # P[h*128 + o128, d, f] -> [d, c, o128]
            wh[h, gj] = np.moveaxis(P[h * 128 : (h + 1) * 128][:, dd, f], 0, -1)
    w0 = np.ascontiguousarray(wh[0].reshape(128, NCH * 128)).astype(np.float16)
    w1 = np.ascontiguousarray(wh[1].reshape(128, NCH * 128)).astype(np.float16)
    # rotation indicator: chunk c rows p=(gj,d), g=2c+1+gj:
    # indc2[k, c*128+p] = s * ([k == d] + [k == (d+g)%64]), s = sqrt(1/2)
    # only for the g=32 rows (each {d, d+32} pair is covered twice)
    k = np.arange(D)[:, None, None, None]
    cc = np.arange(NCH)[None, :, None, None]
    gj = np.arange(2)[None, None, :, None]
    dn = np.arange(D)[None, None, None, :]
    g = 2 * cc + 1 + gj
    ind = (k == dn).astype(np.float32) + (k == (dn + g) % D)
    ind *= np.where(g == 32, np.sqrt(0.5), 1.0)
    indc2 = np.ascontiguousarray(ind.reshape(D, NCH * 128)).astype(np.float16)
    # tiny linear-term prep: w = B^T c, v = B w, q3 = w.w, r = P 1,
    # Pdd = diag(P)
    w_ = np.einsum("ofe,of->oe", betas, c)
    v = np.einsum("ode,oe->od", betas, w_)
    q3 = np.einsum("oe,oe->o", w_, w_)
    r = P.sum(axis=2)  # [o, d]
    pdd = np.einsum("odd->od", P)
    waug1 = np.concatenate([-2.0 * v.T, (-r + 2.0 * pdd).T], axis=0).astype(
        np.float16
    )
    q3b = np.ascontiguousarray((-q3).reshape(2, 128).T).astype(np.float32)
    xind_shards = [
        np.ascontiguousarray(
            np.hstack([x[i * BSH : (i + 1) * BSH].T.astype(np.float16), indc2])
        )
        for i in range(NCORES)
    ]
    return xind_shards, w0, w1, waug1, q3b


def _run(x, centers, betas, trace=False):
    nc, (xinda, w0a, w1a, waug1a, q3ba, outT) = _build()
    xind_shards, w0, w1, waug1, q3b = _host_prep(x, centers, betas)
    in_maps = [
        {
            xinda.name: xind_shards[i],
            w0a.name: w0,
            w1a.name: w1,
            waug1a.name: waug1,
            q3ba.name: q3b,
        }
        for i in range(NCORES)
    ]
    res = bass_utils.run_bass_kernel_spmd(
        nc, in_maps, core_ids=list(range(NCORES)), trace=trace
    )
    out = np.concatenate(
        [np.asarray(res.results[i][outT.name]).T for i in range(NCORES)],
        axis=0,
    )
    return out.astype(np.float32), res


def kernel(x, centers, betas):
    out, _ = _run(x, centers, betas, trace=False)
    return out
